# revision 21
# baseline (speedup 1.0000x reference)
"""Trainium2 Bass kernel for nn_MultiHeadAttention_7413113553038.

Sharding: 8 cores = (batch b in {0,1}) x (query block of 512). Each core
computes all 4 heads of attention for its 512 queries against the full 2048
keys of its batch, plus the output projection, residual add and LayerNorm for
its rows. No collectives needed.

Per-core strategy:
  - Host passes X_Q^T (block), X_K^T, X_V^T (pre-transposed + rolled), weights
    in fp16, and precomputed multiplicative Gaussian-bias band tables
    E = exp(bias) (neutral value 1 outside the causal band).
  - Q^T/K^T computed in [d, seq] layout directly (lhsT = W, rhs = X^T).
  - scores computed transposed: sT[k, q] = K Q^T (contraction d=64, heads at
    partition bases 0/64 so head pairs use distinct PE row groups).
  - p = exp(sT) with NO max-subtraction (scores are O(6), exp safe in f32),
    then p *= E_slice on the 6 k-chunk slots covering the causal band
    (X_K/X_V are rolled by q0-256 so the band sits on static slots 0..5).
  - V is augmented with a ones-column so ctxT = V_aug.T @ p yields the
    softmax denominator Z as psum row 64 for free.
  - 1/Z: Z rows are scattered to 128 partitions with tiny PE transposes,
    reciprocal'd at [128,16], transposed back and broadcast across partitions
    with selector matmuls; ctxT is scaled before the fc projection.
  - ctxT [dm, q] is exactly the lhsT layout the fc matmul needs; LayerNorm
    via bn_stats/bn_aggr; fc psum + residual + LN fused per 128-row chunk.
"""

import numpy as np

N_HEADS = 4
D_K = 64
B = 2
S = 2048
F = 256
QB = 512  # queries per core
P = 128
KC = S // P  # 16 k-chunks
SIGMA_HS = (5.0, 10.0, 20.0, 40.0)
LN_EPS = 1e-5
N_CORES = 8

_CACHE = {}


def _gauss_tables():
    """Multiplicative Toeplitz band tables E[h,i,m] = exp(g_h(delta)) for the
    causal Gaussian bias, in transposed-score layout (delta = q - k =
    off_t + j - i). Neutral value is 1 (g=0 outside the causal band).

    E01 covers slots 0,1 (slice starts 128, 0):  delta = m - i + 128
    E25 covers slots 2..5 (starts 384,256,128,0): delta = m - i - 384
    g_h(d) = exp(-d^2 / (2 sigma_h^2)) for d >= 0 else 0.
    """
    i = np.arange(P, dtype=np.float64)[None, :, None]
    sig = np.asarray(SIGMA_HS, dtype=np.float64)[:, None, None]

    m01 = np.arange(640, dtype=np.float64)[None, None, :]
    d01 = m01 - i + 128.0
    g01 = np.where(d01 >= 0, np.exp(-(d01 ** 2) / (2 * sig ** 2)), 0.0)

    m25 = np.arange(896, dtype=np.float64)[None, None, :]
    d25 = m25 - i - 384.0
    g25 = np.where(d25 >= 0, np.exp(-(d25 ** 2) / (2 * sig ** 2)), 0.0)
    return (
        np.exp(g01).astype(np.float16),
        np.exp(g25).astype(np.float16),
    )


def _build_program():
    import concourse.bass as bass  # noqa: F401
    import concourse.tile as tile
    from concourse import bacc, mybir
    from concourse.masks import make_identity

    f32 = mybir.dt.float32
    f16 = mybir.dt.float16
    AF = mybir.ActivationFunctionType
    ALU = mybir.AluOpType

    nc = bacc.Bacc("TRN2", target_bir_lowering=False, debug=False)

    xqt = nc.dram_tensor("xqt", [F, QB], f16, kind="ExternalInput").ap()
    res = nc.dram_tensor("res", [QB, F], f32, kind="ExternalInput").ap()
    xkt = nc.dram_tensor("xkt", [F, S], f16, kind="ExternalInput").ap()
    xvt = nc.dram_tensor("xvt", [F, S], f16, kind="ExternalInput").ap()
    wq = nc.dram_tensor("wq8", [F, F], f16, kind="ExternalInput").ap()
    wk = nc.dram_tensor("wk", [F, F], f16, kind="ExternalInput").ap()
    wv = nc.dram_tensor("wv", [F, F], f16, kind="ExternalInput").ap()
    wfc = nc.dram_tensor("wfc", [F, F], f16, kind="ExternalInput").ap()
    e01 = nc.dram_tensor("e01", [N_HEADS, P, 640], f16, kind="ExternalInput").ap()
    e25 = nc.dram_tensor("e25", [N_HEADS, P, 896], f16, kind="ExternalInput").ap()
    out = nc.dram_tensor("out", [QB, F], f32, kind="ExternalOutput").ap()

    with tile.TileContext(nc) as tc:
        with (
            tc.tile_pool(name="wpool", bufs=1) as wpool,
            tc.tile_pool(name="xpool", bufs=1) as xpool,
            tc.tile_pool(name="proj", bufs=1) as proj,
            tc.tile_pool(name="mmps", bufs=2, space="PSUM") as mmps,
            tc.tile_pool(name="spsum", bufs=2, space="PSUM") as spsum,
            tc.tile_pool(name="cpsum", bufs=2, space="PSUM") as cpsum,
            tc.tile_pool(name="ptpool", bufs=3) as ptpool,
            tc.tile_pool(name="opool", bufs=3) as opool,
        ):
            # ---- load inputs: small tensors first, big ones in chunks so the
            # projections can start while the rest streams in ----
            xqt_sb = xpool.tile([P, 2, QB], f16, tag="xqt")
            nc.sync.dma_start(xqt_sb, xqt.rearrange("(c p) q -> p c q", p=P))
            wq_sb = wpool.tile([P, 2, F], f16, tag="wq")
            nc.sync.dma_start(wq_sb, wq.rearrange("(c p) o -> p c o", p=P))
            wk_sb = wpool.tile([P, 2, F], f16, tag="wk")
            nc.sync.dma_start(wk_sb, wk.rearrange("(c p) o -> p c o", p=P))
            wv_sb = wpool.tile([P, 2, F], f16, tag="wv")
            nc.sync.dma_start(wv_sb, wv.rearrange("(c p) o -> p c o", p=P))
            wfc_sb = wpool.tile([P, 2, F], f16, tag="wfc")
            nc.sync.dma_start(wfc_sb, wfc.rearrange("(c p) o -> p c o", p=P))
            e01_sb = wpool.tile([P, N_HEADS, 640], f16, tag="e01")
            nc.gpsimd.dma_start(e01_sb, e01.rearrange("h p m -> p h m"))
            e25_sb = wpool.tile([P, N_HEADS, 896], f16, tag="e25")
            nc.gpsimd.dma_start(e25_sb, e25.rearrange("h p m -> p h m"))

            xkt_b = []
            xvt_b = []
            for nb in range(4):
                kb = xpool.tile([P, 2, 512], f16, tag=f"xkt{nb}", name=f"xkt{nb}")
                nc.sync.dma_start(
                    kb,
                    xkt[:, nb * 512:(nb + 1) * 512].rearrange(
                        "(c p) k -> p c k", p=P
                    ),
                )
                xkt_b.append(kb)
                vb = xpool.tile([P, 2, 512], f16, tag=f"xvt{nb}", name=f"xvt{nb}")
                nc.sync.dma_start(
                    vb,
                    xvt[:, nb * 512:(nb + 1) * 512].rearrange(
                        "(c p) k -> p c k", p=P
                    ),
                )
                xvt_b.append(vb)

            ident_f = wpool.tile([P, P], f32, tag="identf")
            make_identity(nc, ident_f)
            ones_t = wpool.tile([P, P], f32, tag="ones")
            nc.vector.memset(ones_t, 1.0)
            sel = wpool.tile([4, N_HEADS, P], f16, tag="sel")
            for h in range(N_HEADS):
                nc.vector.tensor_scalar_mul(
                    sel[0:4, h, :], ones_t[0:4, :], ident_f[0:4, h:h + 1]
                )
            eps_t = wpool.tile([P, 1], f32, tag="eps")
            nc.vector.memset(eps_t, LN_EPS)

            # ---- projections ----
            qt_sb = proj.tile([P, 2, QB], f16, tag="qt")
            kt_sb = proj.tile([P, 2, S], f16, tag="kt")
            v_sb = proj.tile([P, KC, N_HEADS, 65], f16, tag="v")
            ctx_sb = proj.tile([P, 2, QB], f16, tag="ctx")
            ztmp_z = proj.tile([P, N_HEADS, QB], f32, tag="z")

            for g in range(2):
                ps = mmps.tile([P, 512], f32, tag="mm")
                for c in range(2):
                    nc.tensor.matmul(
                        ps,
                        wq_sb[:, c, g * P:(g + 1) * P],
                        xqt_sb[:, c, :],
                        start=(c == 0),
                        stop=(c == 1),
                    )
                nc.vector.tensor_copy(qt_sb[:, g, :], ps)

            for nb in range(4):
                for g in range(2):
                    ps = mmps.tile([P, 512], f32, tag="mm")
                    for c in range(2):
                        nc.tensor.matmul(
                            ps,
                            wk_sb[:, c, g * P:(g + 1) * P],
                            xkt_b[nb][:, c, :],
                            start=(c == 0),
                            stop=(c == 1),
                        )
                    nc.vector.tensor_copy(kt_sb[:, g, nb * 512:(nb + 1) * 512], ps)

            for kc in range(KC):
                ps = mmps.tile([P, 512], f32, tag="mm")
                psv = ps[:, :F]
                for c in range(2):
                    nc.tensor.matmul(
                        psv,
                        xvt_b[kc // 4][:, c, (kc % 4) * P:(kc % 4 + 1) * P],
                        wv_sb[:, c, :],
                        start=(c == 0),
                        stop=(c == 1),
                    )
                nc.vector.tensor_copy(
                    v_sb[:, kc, :, 0:64], psv.rearrange("p (h d) -> p h d", h=N_HEADS)
                )
            nc.vector.tensor_copy(
                v_sb[:, :, :, 64:65],
                ones_t[:, 0:KC * N_HEADS].rearrange(
                    "p (kc h one) -> p kc h one", kc=KC, h=N_HEADS, one=1
                ),
            )

            # ---- attention (2 heads per pass) ----
            zt_ps = mmps.tile([P, 512], f32, tag="mm", name="zt_ps")
            for G in ((0, 1), (2, 3)):
                ctxps = [
                    cpsum.tile([P, QB], f32, tag="ctxp", name=f"ctxp{hh}")
                    for hh in G
                ]
                for kc in range(KC):
                    ps = spsum.tile([P, 2 * QB], f32, tag="sc")
                    for hi, h in enumerate(G):
                        g, po = h // 2, (h % 2) * 64
                        nc.tensor.matmul(
                            ps[:, hi * QB:(hi + 1) * QB],
                            kt_sb[po:po + 64, g, kc * P:(kc + 1) * P],
                            qt_sb[po:po + 64, g, :],
                            start=True,
                            stop=True,
                        )
                    pt = ptpool.tile([P, 2 * QB], f16, tag="pt")
                    nc.scalar.activation(pt, ps, AF.Exp)
                    for hi, h in enumerate(G):
                        if kc <= 5:
                            if kc <= 1:
                                esl = e01_sb[:, h, 128 - 128 * kc:640 - 128 * kc]
                            else:
                                s_t = 384 - 128 * (kc - 2)
                                esl = e25_sb[:, h, s_t:s_t + QB]
                            nc.vector.tensor_mul(
                                pt[:, hi * QB:(hi + 1) * QB],
                                pt[:, hi * QB:(hi + 1) * QB],
                                esl,
                            )
                        nc.tensor.matmul(
                            ctxps[hi][0:65, :],
                            v_sb[:, kc, h, 0:65],
                            pt[:, hi * QB:(hi + 1) * QB],
                            start=(kc == 0),
                            stop=(kc == KC - 1),
                        )
                for hi, h in enumerate(G):
                    g, po = h // 2, (h % 2) * 64
                    nc.vector.tensor_copy(ctx_sb[po:po + 64, g, :], ctxps[hi][0:64, :])
                    nc.vector.tensor_copy(ztmp_z[64:65, h, :], ctxps[hi][64:65, :])
                    for qc in range(4):
                        nc.tensor.transpose(
                            zt_ps[:, h * 4 + qc:h * 4 + qc + 1],
                            ztmp_z[64:65, h, qc * P:(qc + 1) * P],
                            ident_f[64:65, 64:65],
                        )

            # ---- normalize ctx by 1/Z ----
            # Z was scattered to [128,16] via tiny PE transposes; exact
            # reciprocal there is cheap, then transpose back per head and
            # broadcast across partitions with selector matmuls.
            ztc = opool.tile([P, 16], f32, tag="ztc")
            nc.vector.tensor_copy(ztc, zt_ps[:, 0:16])
            nc.vector.reciprocal(ztc, ztc)
            rz_ps = mmps.tile([P, 512], f32, tag="mm", name="rz_ps")
            for h in range(N_HEADS):
                nc.tensor.transpose(
                    rz_ps[0:4, h * P:(h + 1) * P],
                    ztc[:, h * 4:(h + 1) * 4],
                    ident_f,
                )
            rz4 = opool.tile([4, N_HEADS, P], f16, tag="rz4")
            nc.vector.tensor_copy(
                rz4, rz_ps[0:4, :].rearrange("p (h j) -> p h j", h=N_HEADS)
            )
            for h in range(N_HEADS):
                g, po = h // 2, (h % 2) * 64
                zb = mmps.tile([P, 512], f32, tag="mm", name="zb")
                for qc in range(4):
                    nc.tensor.matmul(
                        zb[:, qc * P:(qc + 1) * P],
                        sel[0:4, qc, :],
                        rz4[0:4, h, :],
                        start=True,
                        stop=True,
                    )
                nc.vector.tensor_mul(
                    ctx_sb[po:po + 64, g, :],
                    ctx_sb[po:po + 64, g, :],
                    zb[po:po + 64, :],
                )

            # ---- fc + residual + layernorm, per 128-row query chunk ----
            for qc in range(4):
                pso = mmps.tile([P, 512], f32, tag="mm")
                pso = pso[:, :F]
                for g in range(2):
                    nc.tensor.matmul(
                        pso,
                        ctx_sb[:, g, qc * P:(qc + 1) * P],
                        wfc_sb[:, g, :],
                        start=(g == 0),
                        stop=(g == 1),
                    )
                res_t = opool.tile([P, F], f32, tag="res")
                nc.gpsimd.dma_start(res_t, res[qc * P:(qc + 1) * P, :])
                x_t = opool.tile([P, F], f32, tag="x")
                nc.vector.tensor_add(x_t, pso, res_t)
                st = opool.tile([P, 6], f32, tag="st")
                nc.vector.bn_stats(st, x_t)
                mv = opool.tile([P, 2], f32, tag="mv")
                nc.vector.bn_aggr(mv, st)
                nc.scalar.activation(
                    mv[:, 1:2], mv[:, 1:2], AF.Sqrt, bias=eps_t, scale=1.0
                )
                nc.vector.reciprocal(mv[:, 1:2], mv[:, 1:2])
                o_t = opool.tile([P, F], f32, tag="o")
                nc.vector.tensor_scalar(
                    o_t,
                    x_t,
                    mv[:, 0:1],
                    mv[:, 1:2],
                    op0=ALU.subtract,
                    op1=ALU.mult,
                )
                nc.gpsimd.dma_start(out[qc * P:(qc + 1) * P, :], o_t)

    nc.compile()
    return nc


def get_nc():
    if "nc" not in _CACHE:
        _CACHE["nc"] = _build_program()
    return _CACHE["nc"]


def make_in_maps(input_Q, input_K, input_V, W_Q, W_K, W_V, W_fc):
    c16 = lambda a: np.ascontiguousarray(
        np.asarray(a, dtype=np.float32), dtype=np.float16
    )
    e01, e25 = _gauss_tables()
    e01_neutral = np.ones_like(e01)
    wq8 = c16(np.asarray(W_Q, np.float32) / np.float32(np.sqrt(D_K)))
    wk = c16(W_K)
    wv = c16(W_V)
    wfc = c16(W_fc)
    in_maps = []
    for c in range(N_CORES):
        b, qb = divmod(c, 4)
        q0 = qb * QB
        r = (q0 - 256) % S
        xq_blk = np.asarray(input_Q[b][q0:q0 + QB], np.float32)
        xk_rot = np.roll(np.asarray(input_K[b], np.float32), -r, axis=0)
        xv_rot = np.roll(np.asarray(input_V[b], np.float32), -r, axis=0)
        in_maps.append({
            "xqt": c16(xq_blk.T),
            "res": np.ascontiguousarray(xq_blk, dtype=np.float32),
            "xkt": c16(xk_rot.T),
            "xvt": c16(xv_rot.T),
            "wq8": wq8,
            "wk": wk,
            "wv": wv,
            "wfc": wfc,
            "e01": e01_neutral if q0 == 0 else e01,
            "e25": e25,
        })
    return in_maps


def kernel(input_Q, input_K, input_V, W_Q, W_K, W_V, W_fc, attn_mask=None):
    from concourse.bass_utils import run_bass_kernel_spmd

    nc = get_nc()
    in_maps = make_in_maps(input_Q, input_K, input_V, W_Q, W_K, W_V, W_fc)
    res = run_bass_kernel_spmd(nc, in_maps, core_ids=list(range(N_CORES)))
    out = np.empty((B, S, F), dtype=np.float32)
    for c in range(N_CORES):
        b, qb = divmod(c, 4)
        out[b, qb * QB:(qb + 1) * QB, :] = res.results[c]["out"]
    return out


# revision 23
# speedup vs baseline: 1.0237x; 1.0237x over previous
"""Trainium2 Bass kernel for nn_MultiHeadAttention_7413113553038.

Sharding: 8 cores = (batch b in {0,1}) x (query block of 512). Each core
computes all 4 heads of attention for its 512 queries against the full 2048
keys of its batch, plus the output projection, residual add and LayerNorm for
its rows. No collectives needed.

Per-core strategy:
  - Host passes X_Q^T (block), X_K^T, X_V^T (pre-transposed + rolled), weights
    in fp16, and precomputed multiplicative Gaussian-bias band tables
    E = exp(bias) (neutral value 1 outside the causal band).
  - Q^T/K^T computed in [d, seq] layout directly (lhsT = W, rhs = X^T).
  - scores computed transposed: sT[k, q] = K Q^T (contraction d=64, heads at
    partition bases 0/64 so head pairs use distinct PE row groups).
  - p = exp(sT) with NO max-subtraction (scores are O(6), exp safe in f32),
    then p *= E_slice on the 6 k-chunk slots covering the causal band
    (X_K/X_V are rolled by q0-256 so the band sits on static slots 0..5).
  - V is augmented with a ones-column so ctxT = V_aug.T @ p yields the
    softmax denominator Z as psum row 64 for free.
  - 1/Z: Z rows are scattered to 128 partitions with tiny PE transposes,
    reciprocal'd at [128,16], transposed back and broadcast across partitions
    with selector matmuls; ctxT is scaled before the fc projection.
  - ctxT [dm, q] is exactly the lhsT layout the fc matmul needs; LayerNorm
    via bn_stats/bn_aggr; fc psum + residual + LN fused per 128-row chunk.
"""

import numpy as np

N_HEADS = 4
D_K = 64
B = 2
S = 2048
F = 256
QB = 512  # queries per core
P = 128
KC = S // P  # 16 k-chunks
SIGMA_HS = (5.0, 10.0, 20.0, 40.0)
LN_EPS = 1e-5
N_CORES = 8

_CACHE = {}


def _gauss_tables():
    """Multiplicative Toeplitz band tables E[h,i,m] = exp(g_h(delta)) for the
    causal Gaussian bias, in transposed-score layout (delta = q - k =
    off_t + j - i). Neutral value is 1 (g=0 outside the causal band).

    E01 covers slots 0,1 (slice starts 128, 0):  delta = m - i + 128
    E25 covers slots 2..5 (starts 384,256,128,0): delta = m - i - 384
    g_h(d) = exp(-d^2 / (2 sigma_h^2)) for d >= 0 else 0.
    """
    i = np.arange(P, dtype=np.float64)[None, :, None]
    sig = np.asarray(SIGMA_HS, dtype=np.float64)[:, None, None]

    m01 = np.arange(640, dtype=np.float64)[None, None, :]
    d01 = m01 - i + 128.0
    g01 = np.where(d01 >= 0, np.exp(-(d01 ** 2) / (2 * sig ** 2)), 0.0)

    m25 = np.arange(896, dtype=np.float64)[None, None, :]
    d25 = m25 - i - 384.0
    g25 = np.where(d25 >= 0, np.exp(-(d25 ** 2) / (2 * sig ** 2)), 0.0)
    return (
        np.exp(g01).astype(np.float16),
        np.exp(g25).astype(np.float16),
    )


def _build_program():
    import concourse.bass as bass  # noqa: F401
    import concourse.tile as tile
    from concourse import bacc, mybir
    from concourse.masks import make_identity

    f32 = mybir.dt.float32
    f16 = mybir.dt.float16
    AF = mybir.ActivationFunctionType
    ALU = mybir.AluOpType

    nc = bacc.Bacc("TRN2", target_bir_lowering=False, debug=False)

    xqt = nc.dram_tensor("xqt", [F, QB], f16, kind="ExternalInput").ap()
    res = nc.dram_tensor("res", [QB, F], f32, kind="ExternalInput").ap()
    xkt = nc.dram_tensor("xkt", [F, S], f16, kind="ExternalInput").ap()
    xvt = nc.dram_tensor("xvt", [F, S], f16, kind="ExternalInput").ap()
    wq = nc.dram_tensor("wq8", [F, F], f16, kind="ExternalInput").ap()
    wk = nc.dram_tensor("wk", [F, F], f16, kind="ExternalInput").ap()
    wv = nc.dram_tensor("wv", [F, F], f16, kind="ExternalInput").ap()
    wfc = nc.dram_tensor("wfc", [F, F], f16, kind="ExternalInput").ap()
    e01 = nc.dram_tensor("e01", [N_HEADS, P, 640], f16, kind="ExternalInput").ap()
    e25 = nc.dram_tensor("e25", [N_HEADS, P, 896], f16, kind="ExternalInput").ap()
    out = nc.dram_tensor("out", [QB, F], f32, kind="ExternalOutput").ap()

    with tile.TileContext(nc) as tc:
        with (
            tc.tile_pool(name="wpool", bufs=1) as wpool,
            tc.tile_pool(name="xpool", bufs=1) as xpool,
            tc.tile_pool(name="proj", bufs=1) as proj,
            tc.tile_pool(name="mmps", bufs=2, space="PSUM") as mmps,
            tc.tile_pool(name="spsum", bufs=2, space="PSUM") as spsum,
            tc.tile_pool(name="cpsum", bufs=2, space="PSUM") as cpsum,
            tc.tile_pool(name="ptpool", bufs=3) as ptpool,
            tc.tile_pool(name="opool", bufs=3) as opool,
        ):
            # ---- load inputs: small tensors first, big ones in chunks so the
            # projections can start while the rest streams in ----
            xqt_sb = xpool.tile([P, 2, QB], f16, tag="xqt")
            nc.sync.dma_start(xqt_sb, xqt.rearrange("(c p) q -> p c q", p=P))
            wq_sb = wpool.tile([P, 2, F], f16, tag="wq")
            nc.sync.dma_start(wq_sb, wq.rearrange("(c p) o -> p c o", p=P))
            wk_sb = wpool.tile([P, 2, F], f16, tag="wk")
            nc.scalar.dma_start(wk_sb, wk.rearrange("(c p) o -> p c o", p=P))
            wv_sb = wpool.tile([P, 2, F], f16, tag="wv")
            nc.scalar.dma_start(wv_sb, wv.rearrange("(c p) o -> p c o", p=P))
            wfc_sb = wpool.tile([P, 2, F], f16, tag="wfc")
            nc.scalar.dma_start(wfc_sb, wfc.rearrange("(c p) o -> p c o", p=P))
            e01_sb = wpool.tile([P, N_HEADS, 640], f16, tag="e01")
            nc.scalar.dma_start(e01_sb, e01.rearrange("h p m -> p h m"))
            e25_sb = wpool.tile([P, N_HEADS, 896], f16, tag="e25")
            nc.scalar.dma_start(e25_sb, e25.rearrange("h p m -> p h m"))

            xkt_b = []
            xvt_b = []
            for nb in range(4):
                kb = xpool.tile([P, 2, 512], f16, tag=f"xkt{nb}", name=f"xkt{nb}")
                nc.sync.dma_start(
                    kb,
                    xkt[:, nb * 512:(nb + 1) * 512].rearrange(
                        "(c p) k -> p c k", p=P
                    ),
                )
                xkt_b.append(kb)
                vb = xpool.tile([P, 2, 512], f16, tag=f"xvt{nb}", name=f"xvt{nb}")
                nc.sync.dma_start(
                    vb,
                    xvt[:, nb * 512:(nb + 1) * 512].rearrange(
                        "(c p) k -> p c k", p=P
                    ),
                )
                xvt_b.append(vb)

            ident_f = wpool.tile([P, P], f32, tag="identf")
            make_identity(nc, ident_f)
            ones_t = wpool.tile([P, P], f32, tag="ones")
            nc.vector.memset(ones_t, 1.0)
            sel = wpool.tile([4, N_HEADS, P], f16, tag="sel")
            for h in range(N_HEADS):
                nc.vector.tensor_scalar_mul(
                    sel[0:4, h, :], ones_t[0:4, :], ident_f[0:4, h:h + 1]
                )
            eps_t = wpool.tile([P, 1], f32, tag="eps")
            nc.vector.memset(eps_t, LN_EPS)

            # ---- projections (per 512-block so attention can start early) ----
            qt_sb = proj.tile([P, 2, QB], f16, tag="qt")
            kt_b = [
                proj.tile([P, 2, 512], f16, tag=f"kt{nb}", name=f"kt{nb}")
                for nb in range(4)
            ]
            v_b = [
                proj.tile([P, 4, N_HEADS, 65], f16, tag=f"v{nb}", name=f"v{nb}")
                for nb in range(4)
            ]
            ctx_sb = proj.tile([P, 2, QB], f16, tag="ctx")
            ztmp_z = proj.tile([P, N_HEADS, QB], f32, tag="z")

            for g in range(2):
                ps = mmps.tile([P, 512], f32, tag="mm")
                for c in range(2):
                    nc.tensor.matmul(
                        ps,
                        wq_sb[:, c, g * P:(g + 1) * P],
                        xqt_sb[:, c, :],
                        start=(c == 0),
                        stop=(c == 1),
                    )
                nc.vector.tensor_copy(qt_sb[:, g, :], ps)

            for nb in range(4):
                for g in range(2):
                    ps = mmps.tile([P, 512], f32, tag="mm")
                    for c in range(2):
                        nc.tensor.matmul(
                            ps,
                            wk_sb[:, c, g * P:(g + 1) * P],
                            xkt_b[nb][:, c, :],
                            start=(c == 0),
                            stop=(c == 1),
                        )
                    nc.vector.tensor_copy(kt_b[nb][:, g, :], ps)
                for j in range(4):
                    ps = mmps.tile([P, 512], f32, tag="mm")
                    psv = ps[:, :F]
                    for c in range(2):
                        nc.tensor.matmul(
                            psv,
                            xvt_b[nb][:, c, j * P:(j + 1) * P],
                            wv_sb[:, c, :],
                            start=(c == 0),
                            stop=(c == 1),
                        )
                    nc.vector.tensor_copy(
                        v_b[nb][:, j, :, 0:64],
                        psv.rearrange("p (h d) -> p h d", h=N_HEADS),
                    )
                nc.vector.tensor_copy(
                    v_b[nb][:, :, :, 64:65],
                    ones_t[:, 0:4 * N_HEADS].rearrange(
                        "p (j h one) -> p j h one", j=4, h=N_HEADS, one=1
                    ),
                )

            # ---- attention (2 heads per pass) ----
            zt_ps = mmps.tile([P, 512], f32, tag="mm", name="zt_ps")
            for G in ((0, 1), (2, 3)):
                ctxps = [
                    cpsum.tile([P, QB], f32, tag="ctxp", name=f"ctxp{hh}")
                    for hh in G
                ]
                for kc in range(KC):
                    ps = spsum.tile([P, 2 * QB], f32, tag="sc")
                    for hi, h in enumerate(G):
                        g, po = h // 2, (h % 2) * 64
                        nc.tensor.matmul(
                            ps[:, hi * QB:(hi + 1) * QB],
                            kt_b[kc // 4][po:po + 64, g, (kc % 4) * P:(kc % 4 + 1) * P],
                            qt_sb[po:po + 64, g, :],
                            start=True,
                            stop=True,
                        )
                    pt = ptpool.tile([P, 2 * QB], f16, tag="pt")
                    nc.scalar.activation(pt, ps, AF.Exp)
                    for hi, h in enumerate(G):
                        if kc <= 5:
                            if kc <= 1:
                                esl = e01_sb[:, h, 128 - 128 * kc:640 - 128 * kc]
                            else:
                                s_t = 384 - 128 * (kc - 2)
                                esl = e25_sb[:, h, s_t:s_t + QB]
                            nc.vector.tensor_mul(
                                pt[:, hi * QB:(hi + 1) * QB],
                                pt[:, hi * QB:(hi + 1) * QB],
                                esl,
                            )
                        nc.tensor.matmul(
                            ctxps[hi][0:65, :],
                            v_b[kc // 4][:, kc % 4, h, 0:65],
                            pt[:, hi * QB:(hi + 1) * QB],
                            start=(kc == 0),
                            stop=(kc == KC - 1),
                        )
                for hi, h in enumerate(G):
                    g, po = h // 2, (h % 2) * 64
                    nc.vector.tensor_copy(ctx_sb[po:po + 64, g, :], ctxps[hi][0:64, :])
                    nc.vector.tensor_copy(ztmp_z[64:65, h, :], ctxps[hi][64:65, :])
                    for qc in range(4):
                        nc.tensor.transpose(
                            zt_ps[:, h * 4 + qc:h * 4 + qc + 1],
                            ztmp_z[64:65, h, qc * P:(qc + 1) * P],
                            ident_f[64:65, 64:65],
                        )

            # ---- normalize ctx by 1/Z ----
            # Z was scattered to [128,16] via tiny PE transposes; exact
            # reciprocal there is cheap, then transpose back per head and
            # broadcast across partitions with selector matmuls.
            ztc = opool.tile([P, 16], f32, tag="ztc")
            nc.vector.tensor_copy(ztc, zt_ps[:, 0:16])
            nc.vector.reciprocal(ztc, ztc)
            rz_ps = mmps.tile([P, 512], f32, tag="mm", name="rz_ps")
            for h in range(N_HEADS):
                nc.tensor.transpose(
                    rz_ps[0:4, h * P:(h + 1) * P],
                    ztc[:, h * 4:(h + 1) * 4],
                    ident_f,
                )
            rz4 = opool.tile([4, N_HEADS, P], f16, tag="rz4")
            nc.vector.tensor_copy(
                rz4, rz_ps[0:4, :].rearrange("p (h j) -> p h j", h=N_HEADS)
            )
            for h in range(N_HEADS):
                g, po = h // 2, (h % 2) * 64
                zb = mmps.tile([P, 512], f32, tag="mm", name="zb")
                for qc in range(4):
                    nc.tensor.matmul(
                        zb[:, qc * P:(qc + 1) * P],
                        sel[0:4, qc, :],
                        rz4[0:4, h, :],
                        start=True,
                        stop=True,
                    )
                nc.vector.tensor_mul(
                    ctx_sb[po:po + 64, g, :],
                    ctx_sb[po:po + 64, g, :],
                    zb[po:po + 64, :],
                )

            # ---- fc + residual + layernorm, per 128-row query chunk ----
            for qc in range(4):
                pso = mmps.tile([P, 512], f32, tag="mm")
                pso = pso[:, :F]
                for g in range(2):
                    nc.tensor.matmul(
                        pso,
                        ctx_sb[:, g, qc * P:(qc + 1) * P],
                        wfc_sb[:, g, :],
                        start=(g == 0),
                        stop=(g == 1),
                    )
                res_t = opool.tile([P, F], f32, tag="res")
                nc.gpsimd.dma_start(res_t, res[qc * P:(qc + 1) * P, :])
                x_t = opool.tile([P, F], f32, tag="x")
                nc.vector.tensor_add(x_t, pso, res_t)
                st = opool.tile([P, 6], f32, tag="st")
                nc.vector.bn_stats(st, x_t)
                mv = opool.tile([P, 2], f32, tag="mv")
                nc.vector.bn_aggr(mv, st)
                nc.scalar.activation(
                    mv[:, 1:2], mv[:, 1:2], AF.Sqrt, bias=eps_t, scale=1.0
                )
                nc.vector.reciprocal(mv[:, 1:2], mv[:, 1:2])
                o_t = opool.tile([P, F], f32, tag="o")
                nc.vector.tensor_scalar(
                    o_t,
                    x_t,
                    mv[:, 0:1],
                    mv[:, 1:2],
                    op0=ALU.subtract,
                    op1=ALU.mult,
                )
                nc.sync.dma_start(out[qc * P:(qc + 1) * P, :], o_t)

    nc.compile()
    return nc


def get_nc():
    if "nc" not in _CACHE:
        _CACHE["nc"] = _build_program()
    return _CACHE["nc"]


def make_in_maps(input_Q, input_K, input_V, W_Q, W_K, W_V, W_fc):
    c16 = lambda a: np.ascontiguousarray(
        np.asarray(a, dtype=np.float32), dtype=np.float16
    )
    e01, e25 = _gauss_tables()
    e01_neutral = np.ones_like(e01)
    wq8 = c16(np.asarray(W_Q, np.float32) / np.float32(np.sqrt(D_K)))
    wk = c16(W_K)
    wv = c16(W_V)
    wfc = c16(W_fc)
    in_maps = []
    for c in range(N_CORES):
        b, qb = divmod(c, 4)
        q0 = qb * QB
        r = (q0 - 256) % S
        xq_blk = np.asarray(input_Q[b][q0:q0 + QB], np.float32)
        xk_rot = np.roll(np.asarray(input_K[b], np.float32), -r, axis=0)
        xv_rot = np.roll(np.asarray(input_V[b], np.float32), -r, axis=0)
        in_maps.append({
            "xqt": c16(xq_blk.T),
            "res": np.ascontiguousarray(xq_blk, dtype=np.float32),
            "xkt": c16(xk_rot.T),
            "xvt": c16(xv_rot.T),
            "wq8": wq8,
            "wk": wk,
            "wv": wv,
            "wfc": wfc,
            "e01": e01_neutral if q0 == 0 else e01,
            "e25": e25,
        })
    return in_maps


def kernel(input_Q, input_K, input_V, W_Q, W_K, W_V, W_fc, attn_mask=None):
    from concourse.bass_utils import run_bass_kernel_spmd

    nc = get_nc()
    in_maps = make_in_maps(input_Q, input_K, input_V, W_Q, W_K, W_V, W_fc)
    res = run_bass_kernel_spmd(nc, in_maps, core_ids=list(range(N_CORES)))
    out = np.empty((B, S, F), dtype=np.float32)
    for c in range(N_CORES):
        b, qb = divmod(c, 4)
        out[b, qb * QB:(qb + 1) * QB, :] = res.results[c]["out"]
    return out
